# revision 23
# baseline (speedup 1.0000x reference)
"""Trainium2 Bass kernel for nn_BinaryPathEncoder — data-specialized variant.

Same math as kernel.py (6+5+5 bit-chunk table decomposition, 2 matmuls per
position), but the per-position table indices are baked into the program as
static access patterns: the single SPMD program carries 8 specialized
position sections selected at runtime by Switch(partition_id).  This removes
all engine register loads, AP materializations, and the staging gather from
the position loop — the PE stream is pure matmuls.
expm: scaling-and-squaring Taylor (s=3, n=4) with addends folded into
PSUM-accumulated matmuls against pre-scaled identities, 4 chains interleaved.
"""

import contextlib
import numpy as np

DIM = 256
NCORES = 8
P = 128

NAT_E = 63
TRA_E = 65
ENT = 512
NAT_STRIDE = NAT_E * ENT
TRA_STRIDE = TRA_E * ENT

NSTAGE = 4
NSX = 8
NOUT = 8
LAG = 2
EXPM_S = 3
EXPM_N = 4

_NC_CACHE = {}
LAST_RESULTS = None


def _build_nc(npos, core_idx, canon, debug=False):
    """core_idx: per core (slot_qA, slot_qB, qC); canon: (S, g_of, g_start,
    last_pos, G) canonical group structure shared by all cores."""
    from concourse import bass, bacc, mybir

    f32 = mybir.dt.float32
    f32r = mybir.dt.float32r
    f16 = mybir.dt.float16
    Sub = mybir.AluOpType.subtract
    Add = mybir.AluOpType.add

    nc = bacc.Bacc("TRN2", target_bir_lowering=False, debug=debug)

    prims_ext = nc.dram_tensor("prims", [2, DIM, DIM], f32, kind="ExternalInput")
    ident_ext = nc.dram_tensor("ident", [DIM, DIM], f32, kind="ExternalInput")
    assert npos % 4 == 0
    npair = npos // 2
    niter = npair + LAG
    out_ext = nc.dram_tensor("out", [npos, P, 2 * DIM], f16, kind="ExternalOutput")

    with contextlib.ExitStack() as ctx:
        sem = {}
        for name in (["id_sem", "pr0_sem", "pr1_sem",
                      "pe_sem", "dve_sem", "act_sem",
                      "mm1_sem", "mm2_sem", "dvex_sem"]
                     + [f"dma_s{j}" for j in range(NOUT // 2)]):
            sem[name] = ctx.enter_context(nc.semaphore(name))

        rn = ctx.enter_context(nc.sbuf_tensor("rn", [P, NAT_STRIDE], f16))
        rt = ctx.enter_context(nc.sbuf_tensor("rt", [P, TRA_STRIDE], f16))
        pbf = ctx.enter_context(nc.sbuf_tensor("pbf", [P, 2, 2, DIM], f16))
        identf = ctx.enter_context(nc.sbuf_tensor("identf", [P, 2, DIM], f32))
        identr = ctx.enter_context(nc.sbuf_tensor("identr", [P, 2, DIM], f32r))
        i6 = ctx.enter_context(nc.sbuf_tensor("i6", [P, 2, DIM], f32r))
        i2 = ctx.enter_context(nc.sbuf_tensor("i2", [P, 2, DIM], f32r))
        prim = ctx.enter_context(nc.sbuf_tensor("prim", [P, 2, 2, DIM], f32))
        an_ = [ctx.enter_context(nc.sbuf_tensor(f"an{b}", [P, 2, DIM], f32r))
               for b in range(2)]
        ap_ = [ctx.enter_context(nc.sbuf_tensor(f"ap{b}", [P, 2, DIM], f32r))
               for b in range(2)]
        a24 = [ctx.enter_context(nc.sbuf_tensor(f"a24_{b}", [P, 2, DIM], f32r))
               for b in range(2)]
        ye = [ctx.enter_context(nc.sbuf_tensor(f"ye{b}", [P, 2, DIM], f32r))
              for b in range(2)]
        yt = [ctx.enter_context(nc.sbuf_tensor(f"yt{b}", [P, 2, DIM], f32r))
              for b in range(2)]
        stag_x = ctx.enter_context(nc.sbuf_tensor("stag_x", [P, NSX, 2, DIM], f16))
        outb = ctx.enter_context(nc.sbuf_tensor("outb", [P, NOUT, 2, DIM], f16))
        ps = [ctx.enter_context(nc.psum_tensor(f"ps{j}", [P, 2, DIM], f32))
              for j in range(8)]

        ident128 = identf[:, 0, 0:P]

        def ent3(tab, q):
            if tab is rn:
                stride, slot = NAT_STRIDE, q - 1
            else:
                stride, slot = TRA_STRIDE, (0 if q == 1 else q - 63)
            return bass.AP(tab, slot * ENT, [[stride, P], [DIM, 2], [1, DIM]])

        cnt = {k: 0 for k in sem}
        entry_done = {}
        pe_prog, dve_prog, act_prog, sync_prog = [], [], [], []

        # ---------------- DMA in ----------------
        def s_in(s):
            s.dma_start(identf[:, 0, :], ident_ext[0:P, :]).then_inc(sem["id_sem"], 16)
            s.dma_start(identf[:, 1, :], ident_ext[P:2 * P, :]).then_inc(sem["id_sem"], 16)
            for b in range(2):
                s.dma_start(prim[:, b, :, :],
                            bass.AP(prims_ext, b * DIM * DIM,
                                    [[DIM, P], [P * DIM, 2], [1, DIM]]),
                            ).then_inc(sem[f"pr{b}_sem"], 16)
        sync_prog.append(s_in)

        def d_ident(d):
            d.wait_ge(sem["id_sem"], 32)
            d.tensor_copy(ent3(rn, 1), identf[:, :, :])
            d.tensor_copy(ent3(rt, 1), identf[:, :, :])
            d.tensor_copy(identr[:, :, :], identf[:, :, :])
            d.drain()
            d.tensor_scalar_mul(i6[:, :, :], identr[:, :, :], 1.0 / 6.0)
            d.tensor_scalar_mul(i2[:, :, :], identr[:, :, :],
                                0.5).then_inc(sem["dve_sem"], 1)
        dve_prog.append(d_ident)
        cnt["dve_sem"] += 1
        ident_done = cnt["dve_sem"]

        # ---------------- expm ----------------
        inv2s = 1.0 / (2.0 ** EXPM_S)

        for b in range(2):
            def p_tr(t, b=b, wid=ident_done):
                t.wait_ge(sem[f"pr{b}_sem"], 16)
                if b == 0:
                    t.wait_ge(sem["dve_sem"], wid)
                last = None
                for kc in range(2):
                    for mc in range(2):
                        last = t.transpose(
                            out=ps[b][:, kc, mc * P:(mc + 1) * P],
                            in_=prim[:, b, mc, kc * P:(kc + 1) * P],
                            identity=ident128)
                last.then_inc(sem["pe_sem"], 1)
            pe_prog.append(p_tr)
            cnt["pe_sem"] += 1

        prep_done = {}
        for b in range(2):
            def d_prep(d, b=b, w=b + 1):
                d.wait_ge(sem["pe_sem"], w)
                d.tensor_tensor(out=ye[b][:, :, :], in0=ps[b][:, :, :],
                                in1=prim[:, b, :, :], op=Sub)
                d.drain()
                d.tensor_scalar_mul(an_[b][:, :, :], ye[b][:, :, :], inv2s)
                d.tensor_scalar_mul(ap_[b][:, :, :], ye[b][:, :, :], -inv2s)
                d.tensor_scalar_mul(a24[b][:, :, :], ye[b][:, :, :],
                                    -inv2s / 24.0).then_inc(sem["dve_sem"], 1)
            dve_prog.append(d_prep)
            cnt["dve_sem"] += 1
            prep_done[b] = cnt["dve_sem"]

        chains = [(0, 0), (0, 1), (1, 0), (1, 1)]
        ybuf = {(b, s): (ye[b] if s == 0 else yt[b])
                for b in range(2) for s in (0, 1)}
        lhsT_of = {(b, s): (an_[b] if s == 0 else ap_[b])
                   for b in range(2) for s in (0, 1)}
        bank_of = {c: 2 + i for i, c in enumerate(chains)}

        def emit_mm_fused(t, bank, parts, inc=None):
            last = None
            for mc in range(2):
                ops = [(lh, rh, kc) for lh, rh in parts for kc in range(2)]
                for idx, (lh, rh, kc) in enumerate(ops):
                    last = t.matmul(ps[bank][:, mc, :],
                                    lh[:, kc, mc * P:(mc + 1) * P],
                                    rh[:, kc, :],
                                    start=(idx == 0),
                                    stop=(idx == len(ops) - 1))
            if inc is not None:
                last.then_inc(sem[inc], 1)
            return last

        dve_c = cnt["dve_sem"]
        pe_c = cnt["pe_sem"]
        copy_done = {}
        mm_done = {}
        addend = [i6, i2, identr]

        for step in range(3):
            for (b, s) in chains:
                wd = prep_done[b] if step == 0 else copy_done[(b, s)]

                def p_h(t, b=b, s=s, step=step, wd=wd):
                    t.wait_ge(sem["dve_sem"], wd)
                    bank = bank_of[(b, s)]
                    main = ((an_[b], a24[b]) if step == 0
                            else (lhsT_of[(b, s)], ybuf[(b, s)]))
                    emit_mm_fused(t, bank,
                                  [main, (lhsT_of[(b, s)], addend[step])],
                                  inc="pe_sem")
                pe_prog.append(p_h)
                pe_c += 1
                mm_done[(b, s)] = pe_c

                if step < 2:
                    def d_c(d, b=b, s=s, w=pe_c):
                        d.wait_ge(sem["pe_sem"], w)
                        d.tensor_copy(ybuf[(b, s)][:, :, :],
                                      ps[bank_of[(b, s)]][:, :, :],
                                      ).then_inc(sem["dve_sem"], 1)
                else:
                    def d_c(d, b=b, s=s, w=pe_c):
                        d.wait_ge(sem["pe_sem"], w)
                        d.tensor_tensor(out=ybuf[(b, s)][:, :, :],
                                        in0=ps[bank_of[(b, s)]][:, :, :],
                                        in1=identf[:, :, :],
                                        op=Add).then_inc(sem["dve_sem"], 1)
                dve_prog.append(d_c)
                dve_c += 1
                copy_done[(b, s)] = dve_c

        for sq in range(EXPM_S):
            last_sq = (sq == EXPM_S - 1)
            active = [c for c in chains if not (last_sq and c[1] == 1)]
            for (b, s) in active:
                def p_sq(t, b=b, s=s,
                         w=max(copy_done[(b, 0)], copy_done[(b, 1)])):
                    t.wait_ge(sem["dve_sem"], w)
                    emit_mm_fused(t, bank_of[(b, s)],
                                  [(ybuf[(b, 1 - s)], ybuf[(b, s)])],
                                  inc="pe_sem")
                pe_prog.append(p_sq)
                pe_c += 1
                mm_done[(b, s)] = pe_c

            for (b, s) in active:
                dst = (pbf[:, b, :, :] if last_sq
                       else ybuf[(b, s)][:, :, :])
                w = (mm_done[(b, s)] if last_sq
                     else max(mm_done[(b, 0)], mm_done[(b, 1)]))

                def d_sq(d, dst=dst, w=w, bank=bank_of[(b, s)]):
                    d.wait_ge(sem["pe_sem"], w)
                    d.tensor_copy(dst, ps[bank][:, :, :],
                                  ).then_inc(sem["dve_sem"], 1)
                dve_prog.append(d_sq)
                dve_c += 1
                copy_done[(b, s)] = dve_c

        cnt["dve_sem"] = dve_c
        cnt["pe_sem"] = pe_c
        expm_all = max(copy_done[(0, 0)], copy_done[(1, 0)])

        # ---------------- table build ----------------
        build_items = [("n", q) for q in range(2, 64)] + \
                      [("t", q) for q in range(64, 128)]
        bank_owner = {}
        entry_done[("n", 1)] = ("dve_sem", ident_done)
        entry_done[("t", 1)] = ("dve_sem", ident_done)

        for j, (kind, q) in enumerate(build_items):
            bank = j % 8
            b = q & 1
            par = q >> 1

            waits = []
            if j < 8:
                waits.append(("dve_sem", expm_all))
            waits.append(entry_done[("n", par)])
            if bank in bank_owner:
                waits.append(bank_owner[bank])

            def p_build(t, kind=kind, b=b, par=par, bank=bank,
                        waits=tuple(waits)):
                for s_, c_ in waits:
                    t.wait_ge(sem[s_], c_)
                last = None
                for mc in range(2):
                    for kc in range(2):
                        if kind == "n":
                            lhsT = pbf[:, b, kc, mc * P:(mc + 1) * P]
                            rhs = ent3(rn, par)[:, kc, :]
                        else:
                            lhsT = ent3(rn, par)[:, kc, mc * P:(mc + 1) * P]
                            rhs = pbf[:, b, kc, :]
                        last = t.matmul(ps[bank][:, mc, :], lhsT, rhs,
                                        start=(kc == 0), stop=(kc == 1))
                last.then_inc(sem["pe_sem"], 1)
            pe_prog.append(p_build)
            cnt["pe_sem"] += 1

            ceng = "dve_sem" if j % 2 == 0 else "act_sem"
            prog = dve_prog if j % 2 == 0 else act_prog
            tab = rn if kind == "n" else rt

            def x_copy(e, tab=tab, q=q, bank=bank, w=cnt["pe_sem"], ceng=ceng):
                e.wait_ge(sem["pe_sem"], w)
                if ceng == "dve_sem":
                    e.tensor_copy(ent3(tab, q),
                                  ps[bank][:, :, :]).then_inc(sem[ceng], 1)
                else:
                    e.mul(ent3(tab, q),
                          ps[bank][:, :, :], 1.0).then_inc(sem[ceng], 1)
            prog.append(x_copy)
            cnt[ceng] += 1
            entry_done[(kind, q)] = (ceng, cnt[ceng])
            bank_owner[bank] = (ceng, cnt[ceng])

        build_dve = cnt["dve_sem"]
        build_act = cnt["act_sem"]

        # ---------------- positions (per-core specialized, mm1 dedup) -----
        # Canonical group structure (identical on every core): S shared
        # pairs (2 positions, 1 mm1) followed by npos-2S singles.  Per-core
        # data (which table entries each slot uses) lives only in the PE's
        # Switch cases; the DVE cast program is uniform.
        S, g_of, g_start, last_pos, G = canon

        def p_pos(t, bd=build_dve, ba=build_act):
            t.wait_ge(sem["dve_sem"], bd)
            t.wait_ge(sem["act_sem"], ba)
            pid = t.partition_id()
            for c in t.Switch(pid, NCORES):
                (slot_qA, slot_qB, qC) = core_idx[c]
                for k in range(niter):
                    kk = k - LAG
                    if kk >= 0:
                        i0 = 2 * kk
                        t.wait_ge(sem["dvex_sem"], int(g_of[i0 + 1]) + 1)
                        if i0 + 1 >= NSTAGE:
                            t.wait_ge(sem["act_sem"],
                                      ba + i0 + 1 - NSTAGE + 1)
                        for i in (i0, i0 + 1):
                            slot = i % NSTAGE
                            rhs3 = ent3(rn, int(qC[i]))
                            xs = int(g_of[i]) % NSX
                            last = None
                            for mc in range(2):
                                for kc in range(2):
                                    last = t.matmul(
                                        ps[4 + slot][:, mc, :],
                                        stag_x[:, xs, kc, mc * P:(mc + 1) * P],
                                        rhs3[:, kc, :],
                                        start=(kc == 0), stop=(kc == 1))
                            last.then_inc(sem["mm2_sem"], 1)

                    if k < npair:
                        gs = ([k] if k < S else [2 * k - S, 2 * k - S + 1])
                        for g in gs:
                            if g >= NSTAGE:
                                t.wait_ge(sem["dvex_sem"], g - NSTAGE + 1)
                            lhs3 = ent3(rn, int(slot_qB[g]))
                            rhs3 = ent3(rt, int(slot_qA[g]))
                            last = None
                            for mc in range(2):
                                for kc in range(2):
                                    last = t.matmul(
                                        ps[g % NSTAGE][:, mc, :],
                                        lhs3[:, kc, mc * P:(mc + 1) * P],
                                        rhs3[:, kc, :],
                                        start=(kc == 0), stop=(kc == 1))
                            last.then_inc(sem["mm1_sem"], 1)
        pe_prog.append(p_pos)

        def d_pos(d):
            for g in range(G):
                d.wait_ge(sem["mm1_sem"], g + 1)
                if g >= NSX:
                    d.wait_ge(sem["mm2_sem"], int(last_pos[g - NSX]) + 1)
                d.tensor_copy(stag_x[:, g % NSX, :, :],
                              ps[g % NSTAGE][:, :, :],
                              ).then_inc(sem["dvex_sem"], 1)
        dve_prog.append(d_pos)

        def a_pos(a, ba=build_act):
            for i in range(npos):
                slot = i % NSTAGE
                oslot = i % NOUT
                a.wait_ge(sem["mm2_sem"], i + 1)
                k = i // 2
                if k >= NOUT // 2:
                    a.wait_ge(sem[f"dma_s{k % (NOUT // 2)}"],
                              16 * (k // (NOUT // 2)))
                a.mul(outb[:, oslot, :, :],
                      ps[4 + slot][:, :, :], 1.0).then_inc(sem["act_sem"], 1)
        act_prog.append(a_pos)

        def s_pos(s, ba=build_act):
            for k in range(npair):
                oslot = (2 * k) % NOUT
                s.wait_ge(sem["act_sem"], ba + 2 * k + 2)
                dst = bass.AP(out_ext, 2 * k * P * 2 * DIM,
                              [[2 * DIM, P], [P * 2 * DIM, 2], [1, 2 * DIM]])
                s.dma_start(dst, outb[:, oslot:oslot + 2, :, :],
                            ).then_inc(sem[f"dma_s{k % (NOUT // 2)}"], 16)
            for sl in range(NOUT // 2):
                uses = len([k for k in range(npair) if k % (NOUT // 2) == sl])
                if uses:
                    s.wait_ge(sem[f"dma_s{sl}"], 16 * uses)
        sync_prog.append(s_pos)

        # ---------------- emit ----------------
        with nc.Block() as block:
            @block.tensor
            def _(tensor):
                for fn in pe_prog:
                    fn(tensor)

            @block.vector
            def _(vector):
                for fn in dve_prog:
                    fn(vector)

            @block.scalar
            def _(scalar):
                for fn in act_prog:
                    fn(scalar)

            @block.sync
            def _(sync):
                for fn in sync_prog:
                    fn(sync)

    return nc


def _host_indices(u):
    """u: (n,) int64 positions -> (idxA, idxB, idxC) int arrays."""
    u = u.astype(np.int64)
    blen = np.zeros_like(u)
    t = u.copy()
    while np.any(t > 0):
        blen = np.where(t > 0, blen + 1, blen)
        t >>= 1
    k = blen - 1
    tA = np.minimum(k, 6)
    idxA = (1 << tA) + (u & ((1 << tA) - 1))
    tB = np.clip(k - 6, 0, 5)
    idxB = (1 << tB) + ((u >> 6) & ((1 << tB) - 1))
    tC = np.clip(k - 11, 0, 5)
    idxC = (1 << tC) + ((u >> 11) & ((1 << tC) - 1))
    short = u < 64
    idxA = np.where(short, 1, idxA)
    idxB = np.where(short, u, idxB)
    assert idxA.max() < 128 and idxB.max() < 64 and idxC.max() < 64
    assert np.all((idxA == 1) | (idxA >= 64))
    return idxA, idxB, idxC


def _pack(u, npos):
    n = len(u)
    idxA, idxB, idxC = _host_indices(u)
    key = idxA.astype(np.int64) * 64 + idxB
    order = np.argsort(key, kind="stable")
    qA_s, qB_s, qC_s, key_s = idxA[order], idxB[order], idxC[order], key[order]

    # per-core chunks of size <=2 within equal-(idxA,idxB) runs
    core_chunks = []
    for c in range(NCORES):
        sl = slice(c * npos, (c + 1) * npos)
        kk_ = key_s[sl]
        newg = np.ones(npos, bool)
        newg[1:] = kk_[1:] != kk_[:-1]
        starts = np.flatnonzero(newg)
        sizes = np.diff(np.append(starts, npos))
        twos, ones = [], []
        for st, sz in zip(starts, sizes):
            p = st
            while sz >= 2:
                twos.append((p, p + 1))
                p += 2
                sz -= 2
            if sz:
                ones.append(p)
        core_chunks.append((twos, ones))

    # canonical structure: S shared pairs then npos-2S singles (S = min
    # shareable pairs across cores; cores with more split the surplus)
    S = min(len(t) for t, _ in core_chunks)
    G = npos - S
    g_of = np.empty(npos, np.int64)
    g_of[:2 * S] = np.arange(2 * S) // 2
    g_of[2 * S:] = S + np.arange(npos - 2 * S)
    g_start = np.concatenate([2 * np.arange(S), 2 * S + np.arange(npos - 2 * S)])
    last_pos = np.concatenate([2 * np.arange(S) + 1,
                               2 * S + np.arange(npos - 2 * S)])
    canon = (S, g_of, g_start, last_pos, G)

    core_idx = []
    perm = np.empty(n, np.int64)
    for c in range(NCORES):
        twos, ones = core_chunks[c]
        surplus = len(twos) - S
        for (a, b) in twos[S:]:
            ones.extend([a, b])
        twos = twos[:S]
        packed = [p for ab in twos for p in ab] + list(ones)
        assert len(packed) == npos
        packed = np.asarray(packed)
        sl = slice(c * npos, (c + 1) * npos)
        qA, qB, qC = qA_s[sl][packed], qB_s[sl][packed], qC_s[sl][packed]
        slot_qA = qA[g_start]
        slot_qB = qB[g_start]
        core_idx.append((slot_qA, slot_qB, qC))
        perm[c * npos:(c + 1) * npos] = order[sl][packed]
    return core_idx, canon, perm


def kernel(primitives, identity, unique):
    global LAST_RESULTS
    from concourse.bass_utils import run_bass_kernel_spmd

    prims = np.ascontiguousarray(np.asarray(primitives, dtype=np.float32))
    u = np.asarray(unique).astype(np.int64).ravel()
    n = u.shape[0]
    assert n % NCORES == 0
    npos = n // NCORES

    core_idx, canon, perm = _pack(u, npos)
    eye = np.eye(DIM, dtype=np.float32)

    ckey = (npos, u.tobytes())
    if ckey not in _NC_CACHE:
        nc = _build_nc(npos, core_idx, canon)
        nc.compile()
        _NC_CACHE.clear()
        _NC_CACHE[ckey] = nc
    nc = _NC_CACHE[ckey]

    in_maps = [{"prims": prims, "ident": eye} for _ in range(NCORES)]

    import os
    trace_dir = os.environ.get("KERNEL_TRACE_DIR")
    res = run_bass_kernel_spmd(nc, in_maps, core_ids=list(range(NCORES)),
                               tmpdir=trace_dir)
    LAST_RESULTS = res

    parts = []
    for c in range(NCORES):
        o = np.asarray(res.results[c]["out"])
        o = o.reshape(npos, P, 2, DIM).transpose(0, 2, 1, 3)
        parts.append(o.reshape(npos, DIM, DIM).astype(np.float32))
    out = np.empty((n, DIM, DIM), np.float32)
    out[perm] = np.concatenate(parts, axis=0)

    ident = np.asarray(identity, dtype=np.float32)[0]
    if not np.allclose(ident, np.eye(DIM, dtype=np.float32)):
        out = np.einsum("ij,njk->nik", ident, out).astype(np.float32)
    return out
